# revision 27
# baseline (speedup 1.0000x reference)
"""Trainium2 Bass kernel for CompositionalPhoneticsModel (segment_reduce).

Computation (reference):
    phone   = einsum('bth,hp->btp', enc_output, feature2phone) / sqrt(H)
    allo    = where(mapping>0, phone[:,:,None,:]*mapping, -inf)   # mapping is 0/1
    phoneme = max(allo, axis=-1)                                  # masked segment max
    out     = log_softmax(phoneme, axis=2)

Device strategy (8 NeuronCores, data-parallel over the B*T=8192 rows):
  * Host gathers feature2phone columns into segment-contiguous order
    (phones in 2 segments get duplicated columns), pads every segment to a
    length in {4,6,8,10} (512 matmul columns exactly = one PSUM bank, and
    only 4 strided DVE reduce_max ops per row tile), and folds in the
    1/sqrt(H) scale.  The device phoneme order is a permutation of 0..95;
    max/logsumexp are permutation-invariant so the host un-permutes at the
    end.
  * DMA on TRN2 is descriptor-limited per queue (~128-desc transfer = 1.3-2us
    regardless of bytes).  Three queues run in parallel: the two HWDGE
    queues (Sync/Scalar) carry the weights + big enc tiles as partition
    halves, and the GpSimd SWDGE queue — whose software descriptor
    generation coalesces multiple partitions per descriptor — carries the
    small enc tiles and all outputs.
  * enc is pre-cast to bf16 and pre-interleaved per row-tile as
    [128, NH, rows] (chunk-major) so per-partition lines are contiguous.
  * The PE p-state ramps with CONTINUOUS use (~3.7us to full speed; an idle
    gap resets it).  A stream of small warmup matmuls keeps the PE busy
    from kernel start until the real weights arrive.
  * Row tiles (1,2,2,2,1)*128 rows: a small first tile starts the DVE's
    segment-max pipeline ~1.5us earlier, a small last tile halves the
    exposed tail chain.
  * log-softmax without max-subtraction (phone logits are ~N(0,1); exp fits
    fp32 comfortably): Exp on ScalarE (bf16 out -> 2x DVE row sums), Ln and
    negate on ScalarE, final x - lse as ScalarE Identity activation with
    per-partition bias (Identity/Exp/Ln share one activation table).
"""

from contextlib import ExitStack

import numpy as np
import ml_dtypes

import concourse.bass as bass
import concourse.bacc as bacc
import concourse.tile as tile
from concourse import mybir
from concourse.bass_utils import run_bass_kernel_spmd

B, T, H = 8, 1024, 640
N_PHONEME, N_PHONE = 96, 230
N_CORES = 8
ROWS = B * T
RC = ROWS // N_CORES          # rows per core
NH = H // 128                 # contraction chunks
NB = RC // 128                # 128-row blocks per core
TILE_RT = (1, 2, 2, 2, 1)     # row blocks per tile (small head + tail tiles)
NTILES = len(TILE_RT)
NWARM = 40                    # PE-ramp warmup matmuls (128 cols each)
BF16 = ml_dtypes.bfloat16
PAD_LENGTHS = (4, 6, 8, 10)   # segment lengths after padding


def _structure(mapping: np.ndarray):
    """Segment-contiguous gather order, grouped by padded length (desc).

    Returns (col_ids, groups, perm):
      col_ids: phone index feeding each device matmul column (len NNZ)
      groups:  list of (L, nL, col_off, out_off) — nL segments of length L
               occupy matmul cols [col_off, col_off+nL*L) and device output
               cols [out_off, out_off+nL)
      perm:    perm[j] = original phoneme id of device output column j
    """
    segs = [np.nonzero(mapping[m] > 0)[0] for m in range(N_PHONEME)]
    assert min(len(s) for s in segs) >= 1
    # pad segment lengths up to the next target (repeating a member doesn't
    # change the max): fewer distinct lengths -> fewer DVE reduce ops.
    # Only worthwhile while the matmul width stays within one PSUM bank.
    for targets in (PAD_LENGTHS, (2, 4, 6, 8, 10), None):
        if targets is None:
            padded = segs
            break
        padded = []
        for s in segs:
            t = next(t for t in targets if t >= len(s))
            padded.append(np.concatenate([s, np.full(t - len(s), s[0], s.dtype)]))
        if sum(len(s) for s in padded) <= 512:
            break
    segs = padded
    lengths = np.array([len(s) for s in segs])
    order = np.argsort(-lengths, kind="stable")
    col_ids, groups, perm = [], [], []
    i = 0
    while i < N_PHONEME:
        L = int(lengths[order[i]])
        j = i
        while j < N_PHONEME and lengths[order[j]] == L:
            j += 1
        groups.append((L, j - i, len(col_ids), i))
        for k in range(i, j):
            m = int(order[k])
            col_ids.extend(segs[m].tolist())
            perm.append(m)
        i = j
    return np.array(col_ids, dtype=np.int64), groups, np.array(perm, dtype=np.int64)


def _patch_act_tables():
    """Make Exp and Ln resolve to the same activation-table set.

    bacc's insert_act_table_loads models a single table slot, so a kernel
    alternating Exp/Ln reloads a 1.3us table on every transition.  act_info
    has a joint set ('natural_log_exp_and_others') containing both; keep the
    set list's order/indices intact but strip Exp/Ln from the other sets so
    the pass picks the joint set for both and emits a single load.
    """
    if getattr(bacc, "_act_tables_patched", False):
        return
    from concourse import hw_specs
    orig = hw_specs.get_activation_tables
    act = mybir.ActivationFunctionType

    def patched(module_arch):
        tabs = orig(module_arch)
        joint = [k for k, v in tabs.items() if act.Exp in v and act.Ln in v]
        if not joint:
            return tabs
        j = joint[0]
        return {
            k: (v if k == j else (v - {act.Exp, act.Ln}))
            for k, v in tabs.items()
        }

    bacc.get_activation_tables = patched
    bacc._act_tables_patched = True


def _build_program(nnz: int, groups):
    """Build + compile the per-core Bass program. Returns the Bacc object."""
    _patch_act_tables()
    nc = bacc.Bacc("TRN2", target_bir_lowering=False, debug=False)
    dt = mybir.dt
    act = mybir.ActivationFunctionType
    X = mybir.AxisListType.X

    # enc chunk-major per row tile; element (p, c, t) = enc[row0+t, c*128+p]
    enckA_d = nc.dram_tensor("encka", [3, 128, NH, 256], dt.bfloat16, kind="ExternalInput")
    enckB_d = nc.dram_tensor("enckb", [2, 128, NH, 128], dt.bfloat16, kind="ExternalInput")
    # W interleaved: [128, NH, nnz]; element (p, c, n) = W[c*128+p, n]
    wk_d = nc.dram_tensor("wk", [128, NH, nnz], dt.bfloat16, kind="ExternalInput")
    # out packed: [128, NB, 96]; element (p, b, m) = out[b*128+p, m]
    out_d = nc.dram_tensor("out", [128, NB, N_PHONEME], dt.float32, kind="ExternalOutput")

    # tile i -> source slice: B-tensor holds the two 128-row tiles (0, last)
    esrc = [enckB_d[0], enckA_d[0], enckA_d[1], enckA_d[2], enckB_d[1]]

    with ExitStack() as ctx:
        tc = ctx.enter_context(tile.TileContext(nc))
        wpool = ctx.enter_context(tc.tile_pool(name="wpool", bufs=1))
        epool = ctx.enter_context(tc.tile_pool(name="epool", bufs=1))
        ppool = ctx.enter_context(tc.tile_pool(name="ppool", bufs=1, space="PSUM"))
        spool = ctx.enter_context(tc.tile_pool(name="spool", bufs=2))

        wt = wpool.tile([128, NH, nnz], dt.bfloat16)
        ets = [
            epool.tile([128, NH, 128 * rt], dt.bfloat16, tag=f"et{i}", name=f"et{i}")
            for i, rt in enumerate(TILE_RT)
        ]

        # PE warmup setup first: the memset gating the warmup matmuls goes on
        # the (otherwise idle) Vector queue so warmups start immediately.
        wu = wpool.tile([128, 128], dt.bfloat16)
        nc.vector.memset(wu[:], 0.0)

        # Input DMAs: each of the 16 DMA engines owns 8 partitions at
        # ~25GB/s, so a full-width transfer already uses every engine and
        # partition-splitting buys nothing.  Queue switches cost ~0.5-1us of
        # engine time, so ALL inputs go on ONE queue (Sync HWDGE) full-width
        # in need order; outputs go on the GpSimd SWDGE queue, which only
        # becomes active after the input stream drains.
        # Weights alone on the Sync queue (engines serve it first at full
        # rate), the enc tiles in need order on the Scalar queue, outputs on
        # the GpSimd SWDGE queue.  More aggressive splits all measured worse:
        # DMA-instruction completion (the semaphore the consumer waits on)
        # lags the last data byte by ~1.5us regardless of instruction size,
        # so extra splits just add lags and engine-switch stalls.
        nc.sync.dma_start(wt[:], wk_d[:])
        for i in range(NTILES):
            nc.scalar.dma_start(ets[i][:], esrc[i])

        # PE warmup: small dummy matmuls keep the tensor engine continuously
        # busy (ramping its p-state) until the real weights land.  They write
        # the first tile's PSUM bank; the real c==0 matmul (start=True)
        # overwrites it.
        pss = [
            ppool.tile([128, rt, 512], dt.float32, tag=f"ps{rt}",
                       bufs=3 if rt == 2 else 2, name=f"ps{i}")
            for i, rt in enumerate(TILE_RT)
        ]
        for _ in range(NWARM):
            nc.tensor.matmul(pss[0][:, 0, :128], wu[:], wu[:], start=True, stop=True)

        def mms(i):
            ps, et, rt = pss[i], ets[i], TILE_RT[i]
            # tile 0 iterates chunk-outer so each chunk's matmul only waits
            # for that chunk's weight DMA; later tiles have all weights
            loops = ([(r, c) for c in range(NH) for r in range(rt)] if i == 0
                     else [(r, c) for r in range(rt) for c in range(NH)])
            for r, c in loops:
                nc.tensor.matmul(
                    ps[:, r, :nnz],
                    et[:, c, r * 128:(r + 1) * 128],
                    wt[:, c, :],
                    start=(c == 0),
                    stop=(c == NH - 1),
                )

        def seg_max(i, pmax):
            # segment max: one strided DVE reduce per length group
            ps = pss[i]
            for (L, nL, coff, ooff) in groups:
                src = ps[:, :, coff:coff + nL * L].rearrange(
                    "p r (s l) -> p r s l", l=L)
                nc.vector.reduce_max(pmax[:, :, ooff:ooff + nL], src, axis=X)

        def post_tiles(i):
            rt = TILE_RT[i]
            pmax = spool.tile([128, rt, N_PHONEME], dt.float32,
                              tag=f"pmax{rt}", name=f"pmax{i}")
            ex = spool.tile([128, rt, N_PHONEME], dt.bfloat16, tag=f"ex{rt}",
                            name=f"ex{i}")
            se = spool.tile([128, rt], dt.float32, tag=f"se{rt}", name=f"se{i}")
            lse = spool.tile([128, rt], dt.float32, tag=f"lse{rt}",
                             name=f"lse{i}")
            ott = spool.tile([128, rt, N_PHONEME], dt.float32, tag=f"ott{rt}",
                             name=f"ott{i}")
            return pmax, ex, se, lse, ott

        out_rows = np.cumsum([0] + list(TILE_RT))

        # Tiles 0..N-3: straightforward per-tile pipeline.  log-softmax:
        # exp (bf16 -> 2x DVE row sums) -> lse -> -lse -> x + (-lse) as
        # ScalarE Identity with per-partition bias.  Outs ride the GpSimd
        # SWDGE queue (coalesced descriptors, HWDGE queues stay clear).
        for i in range(NTILES - 2):
            rt = TILE_RT[i]
            mms(i)
            pmax, ex, se, lse, ott = post_tiles(i)
            seg_max(i, pmax)
            nc.scalar.activation(ex[:], pmax[:], act.Exp)
            nc.vector.reduce_sum(se[:], ex[:], axis=X)
            nc.scalar.activation(lse[:], se[:], act.Ln)
            if i == NTILES - 3:
                # this tile's post-chain lands in the tail window: its subs
                # go to the idle GpSimd (slow but fully parallel) so the
                # Scalar queue reaches the last tiles' exp/ln ~1.4us earlier
                for r in range(rt):
                    nc.gpsimd.tensor_scalar_sub(ott[:, r, :], pmax[:, r, :],
                                                lse[:, r:r + 1])
            else:
                nls = spool.tile([128, rt], dt.float32, tag=f"nls{rt}",
                                 name=f"nls{i}")
                nc.scalar.activation(nls[:], lse[:], act.Identity, scale=-1.0)
                for r in range(rt):
                    nc.scalar.activation(ott[:, r, :], pmax[:, r, :],
                                         act.Identity, bias=nls[:, r:r + 1])
            nc.gpsimd.dma_start(
                out_d[:, out_rows[i]:out_rows[i + 1], :], ott[:])

        # Last two tiles: hand-interleaved so the exposed tail is minimal.
        # ScalarE only runs exp/ln (its serial chain gated out DMAs before);
        # all subtractions go to the DVE, earliest-ready first, and the two
        # out desc-gens run in parallel on the idle Sync/Scalar queues.
        a, b = NTILES - 2, NTILES - 1
        pmaxa, exa, sea, lsea, otta = post_tiles(a)
        pmaxb, exb, seb, lseb, ottb = post_tiles(b)
        mms(a)
        seg_max(a, pmaxa)
        mms(b)
        seg_max(b, pmaxb)
        nc.scalar.activation(exa[:], pmaxa[:], act.Exp)
        nc.scalar.activation(exb[:], pmaxb[:], act.Exp)
        nc.vector.reduce_sum(sea[:], exa[:], axis=X)
        nc.vector.reduce_sum(seb[:], exb[:], axis=X)
        nc.scalar.activation(lsea[:], sea[:], act.Ln)
        nc.scalar.activation(lseb[:], seb[:], act.Ln)
        for r in range(TILE_RT[a]):
            nc.vector.tensor_scalar_sub(otta[:, r, :], pmaxa[:, r, :],
                                        lsea[:, r:r + 1])
        nc.vector.tensor_scalar_sub(ottb[:, 0, :], pmaxb[:, 0, :],
                                    lseb[:, 0:1])
        nc.scalar.dma_start(out_d[:, out_rows[a]:out_rows[a + 1], :], otta[:])
        nc.sync.dma_start(out_d[:, out_rows[b]:out_rows[b + 1], :], ottb[:])

    nc.compile()
    return nc


_CACHE: dict = {}


def _get_compiled(mapping: np.ndarray):
    key = mapping.astype(np.float32).tobytes()
    if _CACHE.get("key") != key:
        col_ids, groups, perm = _structure(mapping)
        nc = _build_program(len(col_ids), groups)
        _CACHE.update(key=key, col_ids=col_ids, groups=groups, perm=perm, nc=nc)
    return _CACHE["nc"], _CACHE["col_ids"], _CACHE["perm"]


def _prep_in_maps(enc_output, feature2phone, col_ids):
    scale = np.float32(1.0) / np.sqrt(np.float32(H))
    wg = (feature2phone.astype(np.float32) * scale)[:, col_ids].astype(BF16)
    # [H, nnz] -> [128, NH, nnz]
    wk = np.ascontiguousarray(wg.reshape(NH, 128, -1).transpose(1, 0, 2))
    # enc [ROWS, H] -> per-core chunk-major row blocks [NB, 128, NH, 128]
    e4 = enc_output.astype(BF16).reshape(N_CORES, NB, 128, NH, 128)
    e4 = np.ascontiguousarray(e4.transpose(0, 1, 4, 3, 2))
    in_maps = []
    for cc in range(N_CORES):
        blk = e4[cc]  # [NB, 128, NH, 128]; tiles: (0), (1,2), (3,4), (5,6), (7)
        ea = np.stack([
            np.concatenate([blk[2 * j + 1], blk[2 * j + 2]], axis=2)
            for j in range(3)
        ])  # [3, 128, NH, 256]
        eb = np.stack([blk[0], blk[7]])  # [2, 128, NH, 128]
        in_maps.append({
            "encka": np.ascontiguousarray(ea),
            "enckb": np.ascontiguousarray(eb),
            "wk": wk,
        })
    return in_maps


def run_device(enc_output, feature2phone, mapping, trace=False, **kw):
    """Build/compile (cached), run on the 8 cores, return (output, BassKernelResults)."""
    enc_output = np.asarray(enc_output)
    feature2phone = np.asarray(feature2phone)
    mapping = np.asarray(mapping)
    nc, col_ids, perm = _get_compiled(mapping)
    in_maps = _prep_in_maps(enc_output, feature2phone, col_ids)
    res = run_bass_kernel_spmd(
        nc, in_maps, core_ids=list(range(N_CORES)), trace=trace, **kw
    )
    # device out [128, NB, 96] packed -> rows b*128+p
    dev = np.concatenate(
        [res.results[c]["out"].transpose(1, 0, 2).reshape(RC, N_PHONEME)
         for c in range(N_CORES)],
        axis=0,
    )
    out = np.empty_like(dev)
    out[:, perm] = dev
    return out.reshape(B, T, N_PHONEME).astype(np.float32), res


def kernel(enc_output, feature2phone, mapping):
    out, _ = run_device(enc_output, feature2phone, mapping)
    return out


# revision 28
# speedup vs baseline: 1.0439x; 1.0439x over previous
"""Trainium2 Bass kernel for CompositionalPhoneticsModel (segment_reduce).

Computation (reference):
    phone   = einsum('bth,hp->btp', enc_output, feature2phone) / sqrt(H)
    allo    = where(mapping>0, phone[:,:,None,:]*mapping, -inf)   # mapping is 0/1
    phoneme = max(allo, axis=-1)                                  # masked segment max
    out     = log_softmax(phoneme, axis=2)

Device strategy (8 NeuronCores, data-parallel over the B*T=8192 rows):
  * Host gathers feature2phone columns into segment-contiguous order
    (phones in 2 segments get duplicated columns), pads every segment to a
    length in {4,6,8,10} (512 matmul columns exactly = one PSUM bank, and
    only 4 strided DVE reduce_max ops per row tile), and folds in the
    1/sqrt(H) scale.  The device phoneme order is a permutation of 0..95;
    max/logsumexp are permutation-invariant so the host un-permutes at the
    end.
  * DMA on TRN2 is descriptor-limited per queue (~128-desc transfer = 1.3-2us
    regardless of bytes).  Three queues run in parallel: the two HWDGE
    queues (Sync/Scalar) carry the weights + big enc tiles as partition
    halves, and the GpSimd SWDGE queue — whose software descriptor
    generation coalesces multiple partitions per descriptor — carries the
    small enc tiles and all outputs.
  * enc is pre-cast to bf16 and pre-interleaved per row-tile as
    [128, NH, rows] (chunk-major) so per-partition lines are contiguous.
  * The PE p-state ramps with CONTINUOUS use (~3.7us to full speed; an idle
    gap resets it).  A stream of small warmup matmuls keeps the PE busy
    from kernel start until the real weights arrive.
  * Row tiles (1,2,2,2,1)*128 rows: a small first tile starts the DVE's
    segment-max pipeline ~1.5us earlier, a small last tile halves the
    exposed tail chain.
  * log-softmax without max-subtraction (phone logits are ~N(0,1); exp fits
    fp32 comfortably): Exp on ScalarE (bf16 out -> 2x DVE row sums), Ln and
    negate on ScalarE, final x - lse as ScalarE Identity activation with
    per-partition bias (Identity/Exp/Ln share one activation table).
"""

from contextlib import ExitStack

import numpy as np
import ml_dtypes

import concourse.bass as bass
import concourse.bacc as bacc
import concourse.tile as tile
from concourse import mybir
from concourse.bass_utils import run_bass_kernel_spmd

B, T, H = 8, 1024, 640
N_PHONEME, N_PHONE = 96, 230
N_CORES = 8
ROWS = B * T
RC = ROWS // N_CORES          # rows per core
NH = H // 128                 # contraction chunks
NB = RC // 128                # 128-row blocks per core
TILE_RT = (1, 2, 2, 2, 1)     # row blocks per tile (small head + tail tiles)
NTILES = len(TILE_RT)
NWARM = 40                    # PE-ramp warmup matmuls (128 cols each)
BF16 = ml_dtypes.bfloat16
PAD_LENGTHS = (4, 6, 8, 10)   # segment lengths after padding


def _structure(mapping: np.ndarray):
    """Segment-contiguous gather order, grouped by padded length (desc).

    Returns (col_ids, groups, perm):
      col_ids: phone index feeding each device matmul column (len NNZ)
      groups:  list of (L, nL, col_off, out_off) — nL segments of length L
               occupy matmul cols [col_off, col_off+nL*L) and device output
               cols [out_off, out_off+nL)
      perm:    perm[j] = original phoneme id of device output column j
    """
    segs = [np.nonzero(mapping[m] > 0)[0] for m in range(N_PHONEME)]
    assert min(len(s) for s in segs) >= 1
    # pad segment lengths up to the next target (repeating a member doesn't
    # change the max): fewer distinct lengths -> fewer DVE reduce ops.
    # Only worthwhile while the matmul width stays within one PSUM bank.
    for targets in (PAD_LENGTHS, (2, 4, 6, 8, 10), None):
        if targets is None:
            padded = segs
            break
        padded = []
        for s in segs:
            t = next(t for t in targets if t >= len(s))
            padded.append(np.concatenate([s, np.full(t - len(s), s[0], s.dtype)]))
        if sum(len(s) for s in padded) <= 512:
            break
    segs = padded
    lengths = np.array([len(s) for s in segs])
    order = np.argsort(-lengths, kind="stable")
    col_ids, groups, perm = [], [], []
    i = 0
    while i < N_PHONEME:
        L = int(lengths[order[i]])
        j = i
        while j < N_PHONEME and lengths[order[j]] == L:
            j += 1
        groups.append((L, j - i, len(col_ids), i))
        for k in range(i, j):
            m = int(order[k])
            col_ids.extend(segs[m].tolist())
            perm.append(m)
        i = j
    return np.array(col_ids, dtype=np.int64), groups, np.array(perm, dtype=np.int64)


def _patch_act_tables():
    """Make Exp and Ln resolve to the same activation-table set.

    bacc's insert_act_table_loads models a single table slot, so a kernel
    alternating Exp/Ln reloads a 1.3us table on every transition.  act_info
    has a joint set ('natural_log_exp_and_others') containing both; keep the
    set list's order/indices intact but strip Exp/Ln from the other sets so
    the pass picks the joint set for both and emits a single load.
    """
    if getattr(bacc, "_act_tables_patched", False):
        return
    from concourse import hw_specs
    orig = hw_specs.get_activation_tables
    act = mybir.ActivationFunctionType

    def patched(module_arch):
        tabs = orig(module_arch)
        joint = [k for k, v in tabs.items() if act.Exp in v and act.Ln in v]
        if not joint:
            return tabs
        j = joint[0]
        return {
            k: (v if k == j else (v - {act.Exp, act.Ln}))
            for k, v in tabs.items()
        }

    bacc.get_activation_tables = patched
    bacc._act_tables_patched = True


def _build_program(nnz: int, groups):
    """Build + compile the per-core Bass program. Returns the Bacc object."""
    _patch_act_tables()
    nc = bacc.Bacc("TRN2", target_bir_lowering=False, debug=False)
    dt = mybir.dt
    act = mybir.ActivationFunctionType
    X = mybir.AxisListType.X

    # enc chunk-major per row tile; element (p, c, t) = enc[row0+t, c*128+p]
    enckA_d = nc.dram_tensor("encka", [3, 128, NH, 256], dt.bfloat16, kind="ExternalInput")
    enckB_d = nc.dram_tensor("enckb", [2, 128, NH, 128], dt.bfloat16, kind="ExternalInput")
    # W interleaved: [128, NH, nnz]; element (p, c, n) = W[c*128+p, n]
    wk_d = nc.dram_tensor("wk", [128, NH, nnz], dt.bfloat16, kind="ExternalInput")
    # out packed: [128, NB, 96]; element (p, b, m) = out[b*128+p, m]
    out_d = nc.dram_tensor("out", [128, NB, N_PHONEME], dt.float32, kind="ExternalOutput")

    # tile i -> source slice: B-tensor holds the two 128-row tiles (0, last)
    esrc = [enckB_d[0], enckA_d[0], enckA_d[1], enckA_d[2], enckB_d[1]]

    with ExitStack() as ctx:
        tc = ctx.enter_context(tile.TileContext(nc))
        wpool = ctx.enter_context(tc.tile_pool(name="wpool", bufs=1))
        epool = ctx.enter_context(tc.tile_pool(name="epool", bufs=1))
        ppool = ctx.enter_context(tc.tile_pool(name="ppool", bufs=1, space="PSUM"))
        spool = ctx.enter_context(tc.tile_pool(name="spool", bufs=2))

        wt = wpool.tile([128, NH, nnz], dt.bfloat16)
        ets = [
            epool.tile([128, NH, 128 * rt], dt.bfloat16, tag=f"et{i}", name=f"et{i}")
            for i, rt in enumerate(TILE_RT)
        ]

        # PE warmup setup first: the memset gating the warmup matmuls goes on
        # the (otherwise idle) Vector queue so warmups start immediately.
        wu = wpool.tile([128, 128], dt.bfloat16)
        nc.vector.memset(wu[:], 0.0)

        # Input DMAs: each of the 16 DMA engines owns 8 partitions at
        # ~25GB/s, so a full-width transfer already uses every engine and
        # partition-splitting buys nothing.  Queue switches cost ~0.5-1us of
        # engine time, so ALL inputs go on ONE queue (Sync HWDGE) full-width
        # in need order; outputs go on the GpSimd SWDGE queue, which only
        # becomes active after the input stream drains.
        # Weights alone on the Sync queue (engines serve it first at full
        # rate), the enc tiles in need order on the Scalar queue, outputs on
        # the GpSimd SWDGE queue.  More aggressive splits all measured worse:
        # DMA-instruction completion (the semaphore the consumer waits on)
        # lags the last data byte by ~1.5us regardless of instruction size,
        # so extra splits just add lags and engine-switch stalls.
        nc.sync.dma_start(wt[:], wk_d[:])
        for i in range(NTILES):
            nc.scalar.dma_start(ets[i][:], esrc[i])

        # PE warmup: small dummy matmuls keep the tensor engine continuously
        # busy (ramping its p-state) until the real weights land.  They write
        # the first tile's PSUM bank; the real c==0 matmul (start=True)
        # overwrites it.
        pss = [
            ppool.tile([128, rt, 512], dt.float32, tag=f"ps{rt}",
                       bufs=3 if rt == 2 else 2, name=f"ps{i}")
            for i, rt in enumerate(TILE_RT)
        ]
        for _ in range(NWARM):
            nc.tensor.matmul(pss[0][:, 0, :128], wu[:], wu[:], start=True, stop=True)

        def mms(i):
            ps, et, rt = pss[i], ets[i], TILE_RT[i]
            # tile 0 iterates chunk-outer so each chunk's matmul only waits
            # for that chunk's weight DMA; later tiles have all weights
            loops = ([(r, c) for c in range(NH) for r in range(rt)] if i == 0
                     else [(r, c) for r in range(rt) for c in range(NH)])
            for r, c in loops:
                nc.tensor.matmul(
                    ps[:, r, :nnz],
                    et[:, c, r * 128:(r + 1) * 128],
                    wt[:, c, :],
                    start=(c == 0),
                    stop=(c == NH - 1),
                )

        def seg_max(i, pmax):
            # segment max: one strided DVE reduce per length group
            ps = pss[i]
            for (L, nL, coff, ooff) in groups:
                src = ps[:, :, coff:coff + nL * L].rearrange(
                    "p r (s l) -> p r s l", l=L)
                nc.vector.reduce_max(pmax[:, :, ooff:ooff + nL], src, axis=X)

        def post_tiles(i):
            rt = TILE_RT[i]
            pmax = spool.tile([128, rt, N_PHONEME], dt.float32,
                              tag=f"pmax{rt}", name=f"pmax{i}")
            ex = spool.tile([128, rt, N_PHONEME], dt.bfloat16, tag=f"ex{rt}",
                            name=f"ex{i}")
            se = spool.tile([128, rt], dt.float32, tag=f"se{rt}", name=f"se{i}")
            lse = spool.tile([128, rt], dt.float32, tag=f"lse{rt}",
                             name=f"lse{i}")
            ott = spool.tile([128, rt, N_PHONEME], dt.float32, tag=f"ott{rt}",
                             name=f"ott{i}")
            return pmax, ex, se, lse, ott

        out_rows = np.cumsum([0] + list(TILE_RT))

        # Tiles 0..N-3: straightforward per-tile pipeline.  log-softmax:
        # exp (bf16 -> 2x DVE row sums) -> lse -> -lse -> x + (-lse) as
        # ScalarE Identity with per-partition bias.  Outs ride the GpSimd
        # SWDGE queue (coalesced descriptors, HWDGE queues stay clear).
        for i in range(NTILES - 2):
            rt = TILE_RT[i]
            mms(i)
            pmax, ex, se, lse, ott = post_tiles(i)
            seg_max(i, pmax)
            nc.scalar.activation(ex[:], pmax[:], act.Exp)
            nc.vector.reduce_sum(se[:], ex[:], axis=X)
            nc.scalar.activation(lse[:], se[:], act.Ln)
            nls = spool.tile([128, rt], dt.float32, tag=f"nls{rt}",
                             name=f"nls{i}")
            nc.scalar.activation(nls[:], lse[:], act.Identity, scale=-1.0)
            for r in range(rt):
                nc.scalar.activation(ott[:, r, :], pmax[:, r, :],
                                     act.Identity, bias=nls[:, r:r + 1])
            nc.gpsimd.dma_start(
                out_d[:, out_rows[i]:out_rows[i + 1], :], ott[:])

        # Last two tiles: hand-interleaved so the exposed tail is minimal.
        # ScalarE only runs exp/ln (its serial chain gated out DMAs before);
        # all subtractions go to the DVE, earliest-ready first, and the two
        # out desc-gens run in parallel on the idle Sync/Scalar queues.
        a, b = NTILES - 2, NTILES - 1
        pmaxa, exa, sea, lsea, otta = post_tiles(a)
        pmaxb, exb, seb, lseb, ottb = post_tiles(b)
        mms(a)
        seg_max(a, pmaxa)
        mms(b)
        seg_max(b, pmaxb)
        nc.scalar.activation(exa[:], pmaxa[:], act.Exp)
        nc.scalar.activation(exb[:], pmaxb[:], act.Exp)
        nc.vector.reduce_sum(sea[:], exa[:], axis=X)
        nc.vector.reduce_sum(seb[:], exb[:], axis=X)
        nc.scalar.activation(lsea[:], sea[:], act.Ln)
        nc.scalar.activation(lseb[:], seb[:], act.Ln)
        for r in range(TILE_RT[a]):
            nc.vector.tensor_scalar_sub(otta[:, r, :], pmaxa[:, r, :],
                                        lsea[:, r:r + 1])
        nc.vector.tensor_scalar_sub(ottb[:, 0, :], pmaxb[:, 0, :],
                                    lseb[:, 0:1])
        nc.scalar.dma_start(out_d[:, out_rows[a]:out_rows[a + 1], :], otta[:])
        nc.sync.dma_start(out_d[:, out_rows[b]:out_rows[b + 1], :], ottb[:])

    nc.compile()
    return nc


_CACHE: dict = {}


def _get_compiled(mapping: np.ndarray):
    key = mapping.astype(np.float32).tobytes()
    if _CACHE.get("key") != key:
        col_ids, groups, perm = _structure(mapping)
        nc = _build_program(len(col_ids), groups)
        _CACHE.update(key=key, col_ids=col_ids, groups=groups, perm=perm, nc=nc)
    return _CACHE["nc"], _CACHE["col_ids"], _CACHE["perm"]


def _prep_in_maps(enc_output, feature2phone, col_ids):
    scale = np.float32(1.0) / np.sqrt(np.float32(H))
    wg = (feature2phone.astype(np.float32) * scale)[:, col_ids].astype(BF16)
    # [H, nnz] -> [128, NH, nnz]
    wk = np.ascontiguousarray(wg.reshape(NH, 128, -1).transpose(1, 0, 2))
    # enc [ROWS, H] -> per-core chunk-major row blocks [NB, 128, NH, 128]
    e4 = enc_output.astype(BF16).reshape(N_CORES, NB, 128, NH, 128)
    e4 = np.ascontiguousarray(e4.transpose(0, 1, 4, 3, 2))
    in_maps = []
    for cc in range(N_CORES):
        blk = e4[cc]  # [NB, 128, NH, 128]; tiles: (0), (1,2), (3,4), (5,6), (7)
        ea = np.stack([
            np.concatenate([blk[2 * j + 1], blk[2 * j + 2]], axis=2)
            for j in range(3)
        ])  # [3, 128, NH, 256]
        eb = np.stack([blk[0], blk[7]])  # [2, 128, NH, 128]
        in_maps.append({
            "encka": np.ascontiguousarray(ea),
            "enckb": np.ascontiguousarray(eb),
            "wk": wk,
        })
    return in_maps


def run_device(enc_output, feature2phone, mapping, trace=False, **kw):
    """Build/compile (cached), run on the 8 cores, return (output, BassKernelResults)."""
    enc_output = np.asarray(enc_output)
    feature2phone = np.asarray(feature2phone)
    mapping = np.asarray(mapping)
    nc, col_ids, perm = _get_compiled(mapping)
    in_maps = _prep_in_maps(enc_output, feature2phone, col_ids)
    res = run_bass_kernel_spmd(
        nc, in_maps, core_ids=list(range(N_CORES)), trace=trace, **kw
    )
    # device out [128, NB, 96] packed -> rows b*128+p
    dev = np.concatenate(
        [res.results[c]["out"].transpose(1, 0, 2).reshape(RC, N_PHONEME)
         for c in range(N_CORES)],
        axis=0,
    )
    out = np.empty_like(dev)
    out[:, perm] = dev
    return out.reshape(B, T, N_PHONEME).astype(np.float32), res


def kernel(enc_output, feature2phone, mapping):
    out, _ = run_device(enc_output, feature2phone, mapping)
    return out
